# revision 16
# baseline (speedup 1.0000x reference)
"""DeepSets encoder kernel for 8 Trainium2 NeuronCores.

Architecture (v2, point-major layer 2):
  - phi MLP: Linear(16,256) -> LN -> ReLU -> Linear(256,256) -> LN -> ReLU
    -> Linear(256,128); ragged segment mean + broadcast back to [N,128].
  - LN mean-centering folded into weights on the host (exact).
  - LN rstd factors commute through ReLU/matmul; the eps*var1 + eps^2
    refinement is dropped (worst-case 2e-5 relative, tolerance is 2e-2),
    leaving one per-point scale s = rsqrt(sumsq(x2)/H) applied (together
    with the 1/count mean divisor) inside the segment-sum matmul weights.
  - Layer 2 is computed transposed (points on partitions): per 128-point
    chunk, x2T[pt, h] = a1_chunk^T @ W2.  Per-point variance is then a
    free-dim reduction (fused square+accumulate), and the segment sum is
    an M=1 matmul with lhsT = s-scale column (masked at segment edges).
  - Segment means -> W3 on [64-seg blocks] (transpose + 2 matmuls), then
    per-segment broadcast to the output via 0-stride-source DMA.
  - All matmuls in fp16 (full PE rate); accumulation in fp32 PSUM.
  - Data-parallel across 8 cores at segment granularity.
"""

import dataclasses
import numpy as np

import concourse.bass as bass
import concourse.tile as tile
import concourse.mybir as mybir
from concourse import bacc

AF = mybir.ActivationFunctionType
ALU = mybir.AluOpType
DT = mybir.dt

B = 2000
D_IN = 16
H = 256
D_OUT = 128
EPS = 1e-5
T = 512            # points per tile
SEGBLK = 32        # segments per psum block (PE quadrant granularity)
NSLOTS = 3         # live psum slots for blocks (bases 0/32/64)
NCORES = 8


# ----------------------------------------------------------------------------
# host-side planning
# ----------------------------------------------------------------------------

def _make_plans(counts):
    """Split segments into 8 contiguous shards with ~equal point counts."""
    n = counts.sum()
    starts = np.concatenate([[0], np.cumsum(counts)])
    plans = []
    s0 = 0
    for c in range(NCORES):
        target = (c + 1) * n / NCORES
        if c == NCORES - 1:
            s1 = len(counts)
        else:
            s1 = int(np.searchsorted(starts, target))
            s1 = max(s1, s0 + 1)
        plans.append(dict(s0=s0, s1=s1, p0=int(starts[s0]), p1=int(starts[s1])))
        s0 = s1
    return plans


def _fold(inputs):
    W1 = np.asarray(inputs["W1"], np.float64)
    b1 = np.asarray(inputs["b1"], np.float64)
    g1 = np.asarray(inputs["g1"], np.float64)
    be1 = np.asarray(inputs["be1"], np.float64)
    W2 = np.asarray(inputs["W2"], np.float64)
    b2 = np.asarray(inputs["b2"], np.float64)
    g2 = np.asarray(inputs["g2"], np.float64)
    be2 = np.asarray(inputs["be2"], np.float64)
    W3 = np.asarray(inputs["W3"], np.float64)
    b3 = np.asarray(inputs["b3"], np.float64)

    W1c = W1 - W1.mean(axis=1, keepdims=True)
    b1c = b1 - b1.mean()
    W2c = W2 - W2.mean(axis=1, keepdims=True)
    b2c = b2 - b2.mean()

    if np.abs(b2c).max() > 0 or np.abs(be2).max() > 0 or g2.min() <= 0:
        raise NotImplementedError("nontrivial layer-2 LN affine not supported")

    # layer-1 affine (bias/scale columns, applied pre-relu; exact when the
    # LN1 rstd is commuted out, which requires be1 == 0)
    if np.abs(be1).max() > 0:
        raise NotImplementedError("nontrivial be1 not supported")
    triv1 = np.abs(b1c).max() == 0 and np.abs(g1 - 1).max() == 0

    W1f = (W1c * g1[None, :]).astype(np.float16)            # fold g1 into W1
    bias1 = (g1 * b1c).astype(np.float32)                   # per-feature bias
    # NOTE: folding g1 into W1 changes var1 only (dropped anyway); x2 uses
    # a1 = relu(g1*(z@W1c) + g1*b1c) which matches reference post-commute.

    W2cb = np.zeros((128, 512), np.float16)
    for kc in range(2):
        W2cb[:, 256 * kc:256 * kc + 256] = W2c[128 * kc:128 * kc + 128, :]
    W3g = np.zeros((128, 256), np.float16)
    W3f = W3 * g2[:, None]
    for kc in range(2):
        W3g[:, 128 * kc:128 * kc + 128] = W3f[128 * kc:128 * kc + 128, :]

    return dict(
        W1f=W1f, bias1=bias1, triv1=triv1, W2cb=W2cb, W3g=W3g,
        b3=np.asarray(b3, np.float32),
    )


@dataclasses.dataclass
class CoreProg:
    nc: object
    in_map: dict
    out_name: str
    p0: int
    p1: int


def _build_core(plan, z16, consts):
    s0, s1, p0, p1 = plan["s0"], plan["s1"], plan["p0"], plan["p1"]
    counts = consts["counts"][s0:s1].astype(np.int64)
    npts = p1 - p0
    ntiles = (npts + T - 1) // T
    npad = ntiles * T
    nchunks = npad // 128
    nseg = len(counts)
    bnd = np.concatenate([[0], np.cumsum(counts)])

    # per-point 1/cnt^2-style scale panel: s' = rsqrt(sumsq * cnt^2 / H)
    segp = np.searchsorted(bnd, np.arange(npts), side="right") - 1
    ic2 = np.ones(npad, np.float32)
    ic2[:npts] = (counts[segp].astype(np.float64) ** 2 / H).astype(np.float32)
    ic2_panel = ic2.reshape(nchunks, 128).T.copy()          # [128, nchunks]

    # chunk plans: per chunk, groups keyed by segment block; each group is a
    # list of (seg, mask_idx or None) entries feeding one matmul
    mask_cols = []
    chunk_plan = []
    for cc in range(nchunks):
        lo, hi = cc * 128, min(cc * 128 + 128, npts)
        if lo >= npts:
            chunk_plan.append([])
            continue
        sa = int(np.searchsorted(bnd, lo, side="right") - 1)
        sb = int(np.searchsorted(bnd, hi - 1, side="right") - 1)
        runs = [(s, max(int(bnd[s]), lo) - lo, min(int(bnd[s + 1]), hi) - lo)
                for s in range(sa, sb + 1)]
        interior = len(runs) == 1 and runs[0][1] == 0 and runs[0][2] == 128
        groups = {}
        for (s, a, b) in runs:
            if interior:
                mi = None
            else:
                m = np.zeros(128, np.float16)
                m[a:b] = 1.0
                mask_cols.append(m)
                mi = len(mask_cols) - 1
            groups.setdefault(s // SEGBLK, []).append((s, mi))
        chunk_plan.append(sorted(groups.items()))

    nblocks = (nseg + SEGBLK - 1) // SEGBLK
    lasttile = [int((bnd[min(SEGBLK * (b + 1), nseg)] - 1) // T)
                for b in range(nblocks)]

    zt = np.zeros((16, npad), np.float16)
    zt[:, :npts] = z16[:, p0:p1]

    masks_arr = (np.stack(mask_cols, axis=1) if mask_cols
                 else np.zeros((128, 1), np.float16))

    nc = bacc.Bacc("TRN2", target_bir_lowering=False, debug=False, num_devices=1)

    d = {}
    def din(name, arr, dt_):
        d[name] = np.ascontiguousarray(arr)
        return nc.dram_tensor(name, list(arr.shape), dt_, kind="ExternalInput")

    zt_d = din("zt", zt, DT.float16)
    w1_d = din("w1", consts["W1f"], DT.float16)             # [16, 256]
    w2_d = din("w2", consts["W2cb"], DT.float16)            # [128, 512]
    w3_d = din("w3", consts["W3g"], DT.float16)             # [128, 256]
    eye_d = din("eye", np.eye(32, dtype=np.float32), DT.float32)
    masks_d = din("masks", masks_arr, DT.float16)
    ic2_d = din("ic2", ic2_panel, DT.float32)
    b1_d = din("b1col", consts["bias1"].reshape(2, 128, 1), DT.float32)
    epsb_d = din("epsb", np.full((128, 1), 1e-8, np.float32), DT.float32)
    out_d = nc.dram_tensor("out", [npts, D_OUT], DT.float32, kind="ExternalOutput")

    triv1 = consts["triv1"]

    with tile.TileContext(nc) as tc:
        with (
            tc.tile_pool(name="wp", bufs=1) as wp,
            tc.tile_pool(name="zp", bufs=3) as zp,
            tc.tile_pool(name="a1p", bufs=2) as a1p,
            tc.tile_pool(name="a2p", bufs=2) as a2p,
            tc.tile_pool(name="trp", bufs=2) as trp,
            tc.tile_pool(name="vp", bufs=3) as vp,
            tc.tile_pool(name="mcp", bufs=2) as mcp,
            tc.tile_pool(name="bp", bufs=2) as bp,
            tc.tile_pool(name="ph1", bufs=2, space="PSUM") as ph1,
            tc.tile_pool(name="px2", bufs=1, space="PSUM") as px2,
            tc.tile_pool(name="psg", bufs=1, space="PSUM") as psg,
            tc.tile_pool(name="pbk", bufs=1, space="PSUM") as pbk,
        ):
            # ---- persistent weights / panels ----
            w1 = wp.tile([16, 256], DT.float16, tag="w1")
            nc.sync.dma_start(w1[:], w1_d[:, :])
            w2 = wp.tile([128, 512], DT.float16, tag="w2")
            nc.sync.dma_start(w2[:], w2_d[:, :])
            w3 = wp.tile([128, 256], DT.float16, tag="w3")
            nc.sync.dma_start(w3[:], w3_d[:, :])
            eye32 = wp.tile([32, 32], DT.float32, tag="eye")
            nc.sync.dma_start(eye32[:], eye_d[:, :])
            masks = wp.tile([128, masks_arr.shape[1]], DT.float16, tag="masks")
            nc.sync.dma_start(masks[:], masks_d[:, :])
            ic2_sb = wp.tile([128, nchunks], DT.float32, tag="ic2")
            nc.sync.dma_start(ic2_sb[:], ic2_d[:, :])
            b1c_sb = wp.tile([128, 2], DT.float32, tag="b1c")
            for mh in range(2):
                nc.sync.dma_start(b1c_sb[:, mh:mh + 1], b1_d[mh, :, :])
            epsb = wp.tile([128, 1], DT.float32, tag="epsb")
            nc.sync.dma_start(epsb[:], epsb_d[:, :])

            seg_ps = psg.tile([128, 256], DT.float32, tag="segsum")
            slot_started = [False] * NSLOTS

            def emit_front(t):
                """z load + layer 1 + relu for tile t."""
                zt_t = zp.tile([16, T], DT.float16, tag="zt")
                nc.sync.dma_start(zt_t[:], zt_d[:, t * T:(t + 1) * T])
                h1 = ph1.tile([128, 1024], DT.float32, tag="h1")
                for mh in range(2):
                    nc.tensor.matmul(h1[:, 512 * mh:512 * mh + 512],
                                     w1[:, 128 * mh:128 * mh + 128], zt_t[:],
                                     start=True, stop=True)
                a1 = a1p.tile([128, 1024], DT.float16, tag="a1")
                for mh in range(2):
                    if triv1:
                        nc.scalar.activation(a1[:, 512 * mh:512 * mh + 512],
                                             h1[:, 512 * mh:512 * mh + 512],
                                             AF.Relu)
                    else:
                        nc.scalar.activation(a1[:, 512 * mh:512 * mh + 512],
                                             h1[:, 512 * mh:512 * mh + 512],
                                             AF.Relu, bias=b1c_sb[:, mh:mh + 1])
                return a1

            def emit_mid(t, a1):
                """layer 2 (point-major), variance, scale, relu for tile t."""
                x2 = px2.tile([128, 1024], DT.float32, tag="x2")
                for ptc in range(4):
                    for kc in range(2):
                        nc.tensor.matmul(
                            x2[:, 256 * ptc:256 * ptc + 256],
                            a1[:, 512 * kc + 128 * ptc:512 * kc + 128 * ptc + 128],
                            w2[:, 256 * kc:256 * kc + 256],
                            start=(kc == 0), stop=(kc == 1))
                trash = trp.tile([128, 1024], DT.float16, tag="trash")
                v2 = vp.tile([128, 4], DT.float32, tag="v2")
                for ptc in range(4):
                    nc.scalar.activation(trash[:, 256 * ptc:256 * ptc + 256],
                                         x2[:, 256 * ptc:256 * ptc + 256],
                                         AF.Square,
                                         accum_out=v2[:, ptc:ptc + 1])
                v2c = vp.tile([128, 4], DT.float32, tag="v2c")
                nc.vector.scalar_tensor_tensor(v2c[:], v2[:], 1.0,
                                               ic2_sb[:, 4 * t:4 * t + 4],
                                               ALU.mult, ALU.mult)
                sT = vp.tile([128, 4], DT.float16, tag="sT")
                nc.scalar.activation(sT[:], v2c[:], AF.Abs_reciprocal_sqrt,
                                     bias=epsb[:, 0:1])
                a2 = a2p.tile([128, 1024], DT.float16, tag="a2")
                for hh in range(2):
                    nc.vector.tensor_scalar(a2[:, 512 * hh:512 * hh + 512],
                                            x2[:, 512 * hh:512 * hh + 512],
                                            0.0, None, ALU.max)
                return sT, a2

            def emit_seg(t, sT, a2):
                ngroups = sum(len(chunk_plan[cc])
                              for cc in range(4 * t, 4 * t + 4))
                if not ngroups:
                    return
                mcS = mcp.tile([128, 32 * ngroups], DT.float16, tag="mcS")
                nc.vector.memset(mcS[:], 0.0)
                g = 0
                for j in range(4):
                    cc = 4 * t + j
                    rhs = a2[:, 256 * j:256 * j + 256]
                    for (blk, entries) in chunk_plan[cc]:
                        for (s, mi) in entries:
                            col = 32 * g + (s % SEGBLK)
                            if mi is None:
                                nc.vector.tensor_copy(mcS[:, col:col + 1],
                                                      sT[:, j:j + 1])
                            else:
                                nc.vector.tensor_tensor(mcS[:, col:col + 1],
                                                        masks[:, mi:mi + 1],
                                                        sT[:, j:j + 1],
                                                        ALU.mult)
                        slot = blk % NSLOTS
                        base = 32 * slot
                        nc.tensor.matmul(seg_ps[base:base + 32, :],
                                         mcS[:, 32 * g:32 * g + 32], rhs,
                                         start=not slot_started[slot],
                                         stop=True)
                        slot_started[slot] = True
                        g += 1

            def emit_block(b):
                slot = b % NSLOTS
                r0 = 32 * slot
                mb = bp.tile([32, 256], DT.float32, tag="mb")
                nc.vector.tensor_copy(mb[:], seg_ps[r0:r0 + 32, :])
                slot_started[slot] = False
                blk = pbk.tile([128, 512], DT.float32, tag="blk")
                for hc in range(2):
                    nc.tensor.transpose(blk[:, 32 * hc:32 * hc + 32],
                                        mb[:, 128 * hc:128 * hc + 128], eye32[:])
                mbT = bp.tile([128, 64], DT.float16, tag="mbT")
                nc.vector.tensor_copy(mbT[:], blk[:, 0:64])
                for kc in range(2):
                    nc.tensor.matmul(blk[0:32, 64:192],
                                     mbT[:, 32 * kc:32 * kc + 32],
                                     w3[:, 128 * kc:128 * kc + 128],
                                     start=(kc == 0), stop=(kc == 1))
                emb4 = bp.tile([32, 512], DT.float32, tag="emb4")
                src = blk[0:32, 64:192]
                rep = dataclasses.replace(
                    src, ap=[list(src.ap[0]), [0, 4], list(src.ap[1])])
                nc.vector.tensor_copy(emb4[:], rep)
                lo = SEGBLK * b
                hi = min(nseg, lo + SEGBLK)
                for s in range(lo, hi):
                    r = s % SEGBLK
                    st = int(bnd[s])
                    cnt = int(counts[s])
                    n4 = cnt // 4
                    tail = cnt - 4 * n4
                    if n4:
                        src1 = emb4[r:r + 1, :]
                        srcb = dataclasses.replace(
                            src1, ap=[list(src1.ap[0]), [0, n4],
                                      list(src1.ap[1])])
                        dst = out_d[st:st + 4 * n4, :]
                        dstb = dataclasses.replace(dst, ap=[[512, n4], [1, 512]])
                        nc.sync.dma_start(dstb, srcb)
                    if tail:
                        src1 = emb4[r:r + 1, 0:128]
                        srcb = dataclasses.replace(
                            src1, ap=[list(src1.ap[0]), [0, tail],
                                      list(src1.ap[1])])
                        nc.sync.dma_start(out_d[st + 4 * n4:st + cnt, :], srcb)

            # ---- main emission (software-pipelined front stage) ----
            a1 = emit_front(0)
            done_blocks = 0
            for t in range(ntiles):
                sT, a2 = emit_mid(t, a1)
                if t + 1 < ntiles:
                    a1 = emit_front(t + 1)
                emit_seg(t, sT, a2)
                while done_blocks < nblocks and lasttile[done_blocks] == t:
                    emit_block(done_blocks)
                    done_blocks += 1
            while done_blocks < nblocks:
                emit_block(done_blocks)
                done_blocks += 1

    nc.compile()
    return CoreProg(nc=nc, in_map=d, out_name="out", p0=p0, p1=p1)


# ----------------------------------------------------------------------------
# execution: per-device async dispatch of 8 specialized programs
# ----------------------------------------------------------------------------

def _run_programs(progs):
    import jax
    from concourse import bass2jax

    bass2jax.install_neuronx_cc_hook()
    devices = jax.devices()
    futures = []
    for i, prog in enumerate(progs):
        nc = prog.nc
        in_names, out_names, out_avals, zero_outs = [], [], [], []
        for alloc in nc.m.functions[0].allocations:
            if not isinstance(alloc, mybir.MemoryLocationSet):
                continue
            name = alloc.memorylocations[0].name
            if alloc.kind == "ExternalInput":
                in_names.append(name)
            elif alloc.kind == "ExternalOutput":
                out_names.append(name)
                shape = tuple(alloc.tensor_shape)
                dtype = mybir.dt.np(alloc.dtype)
                out_avals.append(jax.core.ShapedArray(shape, dtype))
                zero_outs.append(np.zeros(shape, dtype))
        n_params = len(in_names)
        all_names = in_names + out_names

        def body(*args, nc=nc, out_avals=tuple(out_avals),
                 all_names=tuple(all_names), out_names=tuple(out_names)):
            outs = bass2jax._bass_exec_p.bind(
                *args, out_avals=out_avals, in_names=all_names,
                out_names=out_names, lowering_input_output_aliases=(),
                sim_require_finite=False, sim_require_nnan=False, nc=nc)
            return tuple(outs)

        donate = tuple(range(n_params, n_params + len(out_names)))
        jitted = jax.jit(body, donate_argnums=donate, keep_unused=True)
        dev = devices[i % len(devices)]
        pid_name = nc.partition_id_tensor.name if nc.partition_id_tensor else None
        in_map = dict(prog.in_map)
        if pid_name is not None and pid_name not in in_map:
            in_map[pid_name] = np.array([[i]], np.uint32)
        args = [jax.device_put(np.ascontiguousarray(in_map[n]), dev)
                for n in in_names]
        args += [jax.device_put(z, dev) for z in zero_outs]
        futures.append((jitted(*args), out_names))
    results = []
    for outs, out_names in futures:
        results.append({n: np.asarray(o) for n, o in zip(out_names, outs)})
    return results


def build_programs(inputs):
    counts = np.asarray(inputs["num_points"]).astype(np.int64)
    consts = _fold(inputs)
    consts["counts"] = counts
    plans = _make_plans(counts)
    z16 = np.ascontiguousarray(
        np.asarray(inputs["z_t"], np.float32).T).astype(np.float16)
    progs = [_build_core(p, z16, consts) for p in plans]
    return progs, consts


def kernel(**inputs):
    progs, consts = build_programs(inputs)
    results = _run_programs(progs)
    out = np.empty((sum(p.p1 - p.p0 for p in progs), D_OUT), np.float32)
    for prog, res in zip(progs, results):
        out[prog.p0:prog.p1] = res[prog.out_name]
    b3 = consts["b3"]
    if np.any(b3):
        out += b3[None, :]
    return out


# revision 20
# speedup vs baseline: 1.5708x; 1.5708x over previous
"""DeepSets encoder kernel for 8 Trainium2 NeuronCores.

Architecture (v2, point-major layer 2):
  - phi MLP: Linear(16,256) -> LN -> ReLU -> Linear(256,256) -> LN -> ReLU
    -> Linear(256,128); ragged segment mean + broadcast back to [N,128].
  - LN mean-centering folded into weights on the host (exact).
  - LN rstd factors commute through ReLU/matmul; the eps*var1 + eps^2
    refinement is dropped (worst-case 2e-5 relative, tolerance is 2e-2),
    leaving one per-point scale s = rsqrt(sumsq(x2)/H) applied (together
    with the 1/count mean divisor) inside the segment-sum matmul weights.
  - Layer 2 is computed transposed (points on partitions): per 128-point
    chunk, x2T[pt, h] = a1_chunk^T @ W2.  Per-point variance is then a
    free-dim reduction (fused square+accumulate), and the segment sum is
    an M=1 matmul with lhsT = s-scale column (masked at segment edges).
  - Segment means -> W3 on [64-seg blocks] (transpose + 2 matmuls), then
    per-segment broadcast to the output via 0-stride-source DMA.
  - All matmuls in fp16 (full PE rate); accumulation in fp32 PSUM.
  - Data-parallel across 8 cores at segment granularity.
"""

import dataclasses
import numpy as np

import concourse.bass as bass
import concourse.tile as tile
import concourse.mybir as mybir
from concourse import bacc

AF = mybir.ActivationFunctionType
ALU = mybir.AluOpType
DT = mybir.dt

B = 2000
D_IN = 16
H = 256
D_OUT = 128
EPS = 1e-5
T = 512            # points per tile
SEGBLK = 32        # segments per psum block (PE quadrant granularity)
NSLOTS = 3         # live psum slots for blocks (bases 0/32/64)
NCORES = 8


# ----------------------------------------------------------------------------
# host-side planning
# ----------------------------------------------------------------------------

def _make_plans(counts):
    """Split segments into 8 contiguous shards with ~equal point counts."""
    n = counts.sum()
    starts = np.concatenate([[0], np.cumsum(counts)])
    plans = []
    s0 = 0
    for c in range(NCORES):
        target = (c + 1) * n / NCORES
        if c == NCORES - 1:
            s1 = len(counts)
        else:
            s1 = int(np.searchsorted(starts, target))
            s1 = max(s1, s0 + 1)
        plans.append(dict(s0=s0, s1=s1, p0=int(starts[s0]), p1=int(starts[s1])))
        s0 = s1
    return plans


def _fold(inputs):
    W1 = np.asarray(inputs["W1"], np.float64)
    b1 = np.asarray(inputs["b1"], np.float64)
    g1 = np.asarray(inputs["g1"], np.float64)
    be1 = np.asarray(inputs["be1"], np.float64)
    W2 = np.asarray(inputs["W2"], np.float64)
    b2 = np.asarray(inputs["b2"], np.float64)
    g2 = np.asarray(inputs["g2"], np.float64)
    be2 = np.asarray(inputs["be2"], np.float64)
    W3 = np.asarray(inputs["W3"], np.float64)
    b3 = np.asarray(inputs["b3"], np.float64)

    W1c = W1 - W1.mean(axis=1, keepdims=True)
    b1c = b1 - b1.mean()
    W2c = W2 - W2.mean(axis=1, keepdims=True)
    b2c = b2 - b2.mean()

    if np.abs(b2c).max() > 0 or np.abs(be2).max() > 0 or g2.min() <= 0:
        raise NotImplementedError("nontrivial layer-2 LN affine not supported")

    # layer-1 affine (bias/scale columns, applied pre-relu; exact when the
    # LN1 rstd is commuted out, which requires be1 == 0)
    if np.abs(be1).max() > 0:
        raise NotImplementedError("nontrivial be1 not supported")
    triv1 = np.abs(b1c).max() == 0 and np.abs(g1 - 1).max() == 0

    W1f = (W1c * g1[None, :]).astype(np.float16)            # fold g1 into W1
    bias1 = (g1 * b1c).astype(np.float32)                   # per-feature bias
    # NOTE: folding g1 into W1 changes var1 only (dropped anyway); x2 uses
    # a1 = relu(g1*(z@W1c) + g1*b1c) which matches reference post-commute.

    W2cb = np.zeros((128, 512), np.float16)
    for kc in range(2):
        W2cb[:, 256 * kc:256 * kc + 256] = W2c[128 * kc:128 * kc + 128, :]
    W3g = np.zeros((128, 256), np.float16)
    W3f = W3 * g2[:, None]
    for kc in range(2):
        W3g[:, 128 * kc:128 * kc + 128] = W3f[128 * kc:128 * kc + 128, :]

    return dict(
        W1f=W1f, bias1=bias1, triv1=triv1, W2cb=W2cb, W3g=W3g,
        b3=np.asarray(b3, np.float32),
    )


@dataclasses.dataclass
class CoreProg:
    nc: object
    in_map: dict
    out_name: str
    p0: int
    p1: int


def _build_core(plan, z16, consts):
    s0, s1, p0, p1 = plan["s0"], plan["s1"], plan["p0"], plan["p1"]
    counts = consts["counts"][s0:s1].astype(np.int64)
    npts = p1 - p0
    ntiles = (npts + T - 1) // T
    npad = ntiles * T
    nchunks = npad // 128
    nseg = len(counts)
    bnd = np.concatenate([[0], np.cumsum(counts)])

    # per-point 1/cnt^2-style scale panel: s' = rsqrt(sumsq * cnt^2 / H)
    segp = np.searchsorted(bnd, np.arange(npts), side="right") - 1
    ic2 = np.ones(npad, np.float32)
    ic2[:npts] = (counts[segp].astype(np.float64) ** 2 / H).astype(np.float32)
    ic2_panel = ic2.reshape(nchunks, 128).T.copy()          # [128, nchunks]

    # chunk plans: per chunk, groups keyed by segment block; each group is a
    # list of (seg, mask_idx or None) entries feeding one matmul
    mask_cols = []
    chunk_plan = []
    for cc in range(nchunks):
        lo, hi = cc * 128, min(cc * 128 + 128, npts)
        if lo >= npts:
            chunk_plan.append([])
            continue
        sa = int(np.searchsorted(bnd, lo, side="right") - 1)
        sb = int(np.searchsorted(bnd, hi - 1, side="right") - 1)
        runs = [(s, max(int(bnd[s]), lo) - lo, min(int(bnd[s + 1]), hi) - lo)
                for s in range(sa, sb + 1)]
        interior = len(runs) == 1 and runs[0][1] == 0 and runs[0][2] == 128
        groups = {}
        for (s, a, b) in runs:
            if interior:
                mi = None
            else:
                m = np.zeros(128, np.float16)
                m[a:b] = 1.0
                mask_cols.append(m)
                mi = len(mask_cols) - 1
            groups.setdefault(s // SEGBLK, []).append((s, mi))
        chunk_plan.append(sorted(groups.items()))

    nblocks = (nseg + SEGBLK - 1) // SEGBLK
    lasttile = [int((bnd[min(SEGBLK * (b + 1), nseg)] - 1) // T)
                for b in range(nblocks)]

    zt = np.zeros((16, npad), np.float16)
    zt[:, :npts] = z16[:, p0:p1]

    masks_arr = (np.stack(mask_cols, axis=1) if mask_cols
                 else np.zeros((128, 1), np.float16))

    nc = bacc.Bacc("TRN2", target_bir_lowering=False, debug=False, num_devices=1)

    d = {}
    def din(name, arr, dt_):
        d[name] = np.ascontiguousarray(arr)
        return nc.dram_tensor(name, list(arr.shape), dt_, kind="ExternalInput")

    zt_d = din("zt", zt, DT.float16)
    w1_d = din("w1", consts["W1f"], DT.float16)             # [16, 256]
    w2_d = din("w2", consts["W2cb"], DT.float16)            # [128, 512]
    w3_d = din("w3", consts["W3g"], DT.float16)             # [128, 256]
    eye_d = din("eye", np.eye(32, dtype=np.float32), DT.float32)
    masks_d = din("masks", masks_arr, DT.float16)
    ic2_d = din("ic2", ic2_panel, DT.float32)
    b1_d = din("b1col", consts["bias1"].reshape(2, 128, 1), DT.float32)
    epsb_d = din("epsb", np.full((128, 1), 1e-8, np.float32), DT.float32)
    out_d = nc.dram_tensor("out", [npts, D_OUT], DT.float32, kind="ExternalOutput")

    triv1 = consts["triv1"]

    with tile.TileContext(nc) as tc:
        with (
            tc.tile_pool(name="wp", bufs=1) as wp,
            tc.tile_pool(name="zp", bufs=3) as zp,
            tc.tile_pool(name="a1p", bufs=2) as a1p,
            tc.tile_pool(name="a2p", bufs=2) as a2p,
            tc.tile_pool(name="trp", bufs=2) as trp,
            tc.tile_pool(name="vp", bufs=3) as vp,
            tc.tile_pool(name="mcp", bufs=2) as mcp,
            tc.tile_pool(name="bp", bufs=2) as bp,
            tc.tile_pool(name="ph1", bufs=2, space="PSUM") as ph1,
            tc.tile_pool(name="px2", bufs=1, space="PSUM") as px2,
            tc.tile_pool(name="psg", bufs=1, space="PSUM") as psg,
            tc.tile_pool(name="pbk", bufs=1, space="PSUM") as pbk,
        ):
            # ---- persistent weights / panels ----
            w1 = wp.tile([16, 256], DT.float16, tag="w1")
            nc.sync.dma_start(w1[:], w1_d[:, :])
            w2 = wp.tile([128, 512], DT.float16, tag="w2")
            nc.sync.dma_start(w2[:], w2_d[:, :])
            w3 = wp.tile([128, 256], DT.float16, tag="w3")
            nc.sync.dma_start(w3[:], w3_d[:, :])
            eye32 = wp.tile([32, 32], DT.float32, tag="eye")
            nc.sync.dma_start(eye32[:], eye_d[:, :])
            masks = wp.tile([128, masks_arr.shape[1]], DT.float16, tag="masks")
            nc.sync.dma_start(masks[:], masks_d[:, :])
            ic2_sb = wp.tile([128, nchunks], DT.float32, tag="ic2")
            nc.sync.dma_start(ic2_sb[:], ic2_d[:, :])
            b1c_sb = wp.tile([128, 2], DT.float32, tag="b1c")
            for mh in range(2):
                nc.sync.dma_start(b1c_sb[:, mh:mh + 1], b1_d[mh, :, :])
            epsb = wp.tile([128, 1], DT.float32, tag="epsb")
            nc.sync.dma_start(epsb[:], epsb_d[:, :])

            seg_ps = psg.tile([128, 256], DT.float32, tag="segsum")
            slot_started = [False] * NSLOTS

            def emit_front(t):
                """z load + layer 1 + relu for tile t."""
                zt_t = zp.tile([16, T], DT.float16, tag="zt")
                nc.sync.dma_start(zt_t[:], zt_d[:, t * T:(t + 1) * T])
                h1 = ph1.tile([128, 1024], DT.float32, tag="h1")
                for mh in range(2):
                    nc.tensor.matmul(h1[:, 512 * mh:512 * mh + 512],
                                     w1[:, 128 * mh:128 * mh + 128], zt_t[:],
                                     start=True, stop=True)
                a1 = a1p.tile([128, 1024], DT.float16, tag="a1")
                for mh in range(2):
                    if triv1:
                        nc.scalar.activation(a1[:, 512 * mh:512 * mh + 512],
                                             h1[:, 512 * mh:512 * mh + 512],
                                             AF.Relu)
                    else:
                        nc.scalar.activation(a1[:, 512 * mh:512 * mh + 512],
                                             h1[:, 512 * mh:512 * mh + 512],
                                             AF.Relu, bias=b1c_sb[:, mh:mh + 1])
                return a1

            def emit_x2(t, a1):
                x2 = px2.tile([128, 1024], DT.float32, tag="x2")
                for ptc in range(4):
                    for kc in range(2):
                        nc.tensor.matmul(
                            x2[:, 256 * ptc:256 * ptc + 256],
                            a1[:, 512 * kc + 128 * ptc:512 * kc + 128 * ptc + 128],
                            w2[:, 256 * kc:256 * kc + 256],
                            start=(kc == 0), stop=(kc == 1))
                return x2

            def emit_mid(t, x2):
                """variance, scale, relu for tile t."""
                trash = trp.tile([128, 1024], DT.float16, tag="trash")
                v2 = vp.tile([128, 4], DT.float32, tag="v2")
                for hh in range(2):
                    nc.scalar.activation(trash[:, 512 * hh:512 * hh + 512],
                                         x2[:, 512 * hh:512 * hh + 512],
                                         AF.Square)
                    tr = trash[:, 512 * hh:512 * hh + 512]
                    tr3 = dataclasses.replace(
                        tr, ap=[list(tr.ap[0]), [256, 2], [1, 256]])
                    nc.vector.tensor_reduce(v2[:, 2 * hh:2 * hh + 2], tr3,
                                            mybir.AxisListType.X, ALU.add)
                v2c = vp.tile([128, 4], DT.float32, tag="v2c")
                nc.vector.scalar_tensor_tensor(v2c[:], v2[:], 1.0,
                                               ic2_sb[:, 4 * t:4 * t + 4],
                                               ALU.mult, ALU.mult)
                sT = vp.tile([128, 4], DT.float16, tag="sT")
                nc.scalar.activation(sT[:], v2c[:], AF.Abs_reciprocal_sqrt,
                                     bias=epsb[:, 0:1])
                a2 = a2p.tile([128, 1024], DT.float16, tag="a2")
                for hh in range(2):
                    nc.vector.tensor_scalar(a2[:, 512 * hh:512 * hh + 512],
                                            x2[:, 512 * hh:512 * hh + 512],
                                            0.0, None, ALU.max)
                return sT, a2

            def emit_seg(t, sT, a2):
                ngroups = sum(len(chunk_plan[cc])
                              for cc in range(4 * t, 4 * t + 4))
                if not ngroups:
                    return
                mcS = mcp.tile([128, 32 * ngroups], DT.float16, tag="mcS")
                nc.vector.memset(mcS[:], 0.0)
                g = 0
                for j in range(4):
                    cc = 4 * t + j
                    rhs = a2[:, 256 * j:256 * j + 256]
                    for (blk, entries) in chunk_plan[cc]:
                        for (s, mi) in entries:
                            col = 32 * g + (s % SEGBLK)
                            if mi is None:
                                nc.vector.tensor_copy(mcS[:, col:col + 1],
                                                      sT[:, j:j + 1])
                            else:
                                nc.vector.tensor_tensor(mcS[:, col:col + 1],
                                                        masks[:, mi:mi + 1],
                                                        sT[:, j:j + 1],
                                                        ALU.mult)
                        slot = blk % NSLOTS
                        base = 32 * slot
                        nc.tensor.matmul(seg_ps[base:base + 32, :],
                                         mcS[:, 32 * g:32 * g + 32], rhs,
                                         start=not slot_started[slot],
                                         stop=True)
                        slot_started[slot] = True
                        g += 1

            def emit_block(b):
                slot = b % NSLOTS
                r0 = 32 * slot
                mb = bp.tile([32, 256], DT.float32, tag="mb")
                nc.vector.tensor_copy(mb[:], seg_ps[r0:r0 + 32, :])
                slot_started[slot] = False
                blk = pbk.tile([128, 512], DT.float32, tag="blk")
                for hc in range(2):
                    nc.tensor.transpose(blk[:, 32 * hc:32 * hc + 32],
                                        mb[:, 128 * hc:128 * hc + 128], eye32[:])
                mbT = bp.tile([128, 64], DT.float16, tag="mbT")
                nc.vector.tensor_copy(mbT[:], blk[:, 0:64])
                for kc in range(2):
                    nc.tensor.matmul(blk[0:32, 64:192],
                                     mbT[:, 32 * kc:32 * kc + 32],
                                     w3[:, 128 * kc:128 * kc + 128],
                                     start=(kc == 0), stop=(kc == 1))
                emb4 = bp.tile([32, 512], DT.float32, tag="emb4")
                src = blk[0:32, 64:192]
                rep = dataclasses.replace(
                    src, ap=[list(src.ap[0]), [0, 4], list(src.ap[1])])
                nc.vector.tensor_copy(emb4[:], rep)
                lo = SEGBLK * b
                hi = min(nseg, lo + SEGBLK)
                for s in range(lo, hi):
                    r = s % SEGBLK
                    st = int(bnd[s])
                    cnt = int(counts[s])
                    n4 = cnt // 4
                    tail = cnt - 4 * n4
                    if n4:
                        src1 = emb4[r:r + 1, :]
                        srcb = dataclasses.replace(
                            src1, ap=[list(src1.ap[0]), [0, n4],
                                      list(src1.ap[1])])
                        dst = out_d[st:st + 4 * n4, :]
                        dstb = dataclasses.replace(dst, ap=[[512, n4], [1, 512]])
                        nc.gpsimd.dma_start(dstb, srcb)
                    if tail:
                        src1 = emb4[r:r + 1, 0:128]
                        srcb = dataclasses.replace(
                            src1, ap=[list(src1.ap[0]), [0, tail],
                                      list(src1.ap[1])])
                        nc.gpsimd.dma_start(out_d[st + 4 * n4:st + cnt, :], srcb)

            # ---- main emission (software-pipelined front stage) ----
            a1 = emit_front(0)
            done_blocks = 0
            for t in range(ntiles):
                x2 = emit_x2(t, a1)
                if t + 1 < ntiles:
                    a1 = emit_front(t + 1)
                sT, a2 = emit_mid(t, x2)
                emit_seg(t, sT, a2)
                while done_blocks < nblocks and lasttile[done_blocks] == t:
                    emit_block(done_blocks)
                    done_blocks += 1
            while done_blocks < nblocks:
                emit_block(done_blocks)
                done_blocks += 1

    nc.compile()
    return CoreProg(nc=nc, in_map=d, out_name="out", p0=p0, p1=p1)


# ----------------------------------------------------------------------------
# execution: per-device async dispatch of 8 specialized programs
# ----------------------------------------------------------------------------

def _run_programs(progs):
    import jax
    from concourse import bass2jax

    bass2jax.install_neuronx_cc_hook()
    devices = jax.devices()
    futures = []
    for i, prog in enumerate(progs):
        nc = prog.nc
        in_names, out_names, out_avals, zero_outs = [], [], [], []
        for alloc in nc.m.functions[0].allocations:
            if not isinstance(alloc, mybir.MemoryLocationSet):
                continue
            name = alloc.memorylocations[0].name
            if alloc.kind == "ExternalInput":
                in_names.append(name)
            elif alloc.kind == "ExternalOutput":
                out_names.append(name)
                shape = tuple(alloc.tensor_shape)
                dtype = mybir.dt.np(alloc.dtype)
                out_avals.append(jax.core.ShapedArray(shape, dtype))
                zero_outs.append(np.zeros(shape, dtype))
        n_params = len(in_names)
        all_names = in_names + out_names

        def body(*args, nc=nc, out_avals=tuple(out_avals),
                 all_names=tuple(all_names), out_names=tuple(out_names)):
            outs = bass2jax._bass_exec_p.bind(
                *args, out_avals=out_avals, in_names=all_names,
                out_names=out_names, lowering_input_output_aliases=(),
                sim_require_finite=False, sim_require_nnan=False, nc=nc)
            return tuple(outs)

        donate = tuple(range(n_params, n_params + len(out_names)))
        jitted = jax.jit(body, donate_argnums=donate, keep_unused=True)
        dev = devices[i % len(devices)]
        pid_name = nc.partition_id_tensor.name if nc.partition_id_tensor else None
        in_map = dict(prog.in_map)
        if pid_name is not None and pid_name not in in_map:
            in_map[pid_name] = np.array([[i]], np.uint32)
        args = [jax.device_put(np.ascontiguousarray(in_map[n]), dev)
                for n in in_names]
        args += [jax.device_put(z, dev) for z in zero_outs]
        futures.append((jitted(*args), out_names))
    results = []
    for outs, out_names in futures:
        results.append({n: np.asarray(o) for n, o in zip(out_names, outs)})
    return results


def build_programs(inputs):
    counts = np.asarray(inputs["num_points"]).astype(np.int64)
    consts = _fold(inputs)
    consts["counts"] = counts
    plans = _make_plans(counts)
    z16 = np.ascontiguousarray(
        np.asarray(inputs["z_t"], np.float32).T).astype(np.float16)
    progs = [_build_core(p, z16, consts) for p in plans]
    return progs, consts


def kernel(**inputs):
    progs, consts = build_programs(inputs)
    results = _run_programs(progs)
    out = np.empty((sum(p.p1 - p.p0 for p in progs), D_OUT), np.float32)
    for prog, res in zip(progs, results):
        out[prog.p0:prog.p1] = res[prog.out_name]
    b3 = consts["b3"]
    if np.any(b3):
        out += b3[None, :]
    return out
